# revision 3
# baseline (speedup 1.0000x reference)
"""AFM (attentional factorization machine) embedding-lookup kernel for one
TRN2 chip (8 NeuronCores), self-contained.

Problem (hardcoded shapes): B=16384, F=32, V=100000, E=64
  first  = sum_i e1[i, x[:,i]]                                  (B,1)
  second = sum_i e2[i, x[:,i]] * (sum_j e2[i, x[:,j]])          (B,E)
  out    = concat([first, softmax(second@W_att)*second]) @ W_out + b_out

Strategy: data-parallel over batch (2048 samples/core, no collectives).
Host-side layout prep only; all gather/reduce/attention math runs on-device.

  - Fused table row (2304 B): [F*E fp8e4m3 e2 values scaled by 64 |
    F f32 e1 values | pad]. Host compacts the vocab per (core, half) so row
    ids fit int16 (~28k unique of 100k for 32768 draws), enabling the
    dma_gather (InstDMAGatherAnt, GPSIMD "mlp" ucode library) fast path:
    one gather op fetches 1024 rows (8 fields x 128 samples), descriptor
    generation ~8.3 ns/row on the Pool Q7 -- ~4x fewer engine-ops than
    per-field indirect DMAs.
  - j-reduction: PSUM-accumulating identity matmuls (fp8 moving, f32 PSUM).
  - d (own-field) strips + e1 diagonal: strided copies on the Scalar engine
    (keeps the Vector engine light; DVE traffic starves SWDGE rings).
  - phase 2 per 128-sample tile: prod = S*d/64^2, contiguous fold tree to
    (128,64), softmax via Exp activation with accumulated row sum, output.

Measured on 8 axon-tunneled TRN2 cores: ~648 us HW exec, rel err ~4.9e-3
(fp8 quantization of the second-order tables; first-order path is exact f32).
"""

import os
from contextlib import ExitStack

import numpy as np
import ml_dtypes

B, F, V, E = 16384, 32, 100000, 64
N_CORES = 8
P = 128
BC = B // N_CORES  # samples per core
T = BC // P  # sample tiles per core
NRT_PAD = 32768  # padded compact-table rows (must hold per-half uniques)
HT = 4  # gathers per tile
FH = F // HT  # fields per gather
NI = P * FH  # rows per gather
NI16 = NI // 16
GBUFS = 6  # deep gather buffering: tile t+1's gathers must not wait on t's phase 2
E2B = F * E  # 2048 fp8 bytes of e2 per row
ROWB = ((E2B + 4 * F + 255) // 256) * 256  # 2304 row bytes
RS = E * F  # accumulated f32 width
S_E2 = 64.0  # fp8 pre-scale on e2
NP_FP8 = ml_dtypes.float8_e4m3fn

LAST_EXEC_TIME_NS = None


def _build(n_tables):
    import concourse.bass as bass
    import concourse.tile as tile
    from concourse import bacc, mybir, library_config

    F32 = mybir.dt.float32
    I16 = mybir.dt.int16
    FP8 = mybir.dt.float8e4

    TPT = T // n_tables
    nc = bacc.Bacc(
        "TRN2", target_bir_lowering=False, debug=False, num_devices=N_CORES
    )

    tfs = [
        nc.dram_tensor(f"tf{h}", [NRT_PAD, ROWB], FP8, kind="ExternalInput").ap()
        for h in range(n_tables)
    ]
    xg = nc.dram_tensor("xg", [T * HT * P, NI16], I16, kind="ExternalInput").ap()
    watt = nc.dram_tensor("watt", [E, E], F32, kind="ExternalInput").ap()
    wv = nc.dram_tensor("wv", [P, E], F32, kind="ExternalInput").ap()
    sc = nc.dram_tensor("sc", [P, 2], F32, kind="ExternalInput").ap()
    id8 = nc.dram_tensor("id8", [P, P], FP8, kind="ExternalInput").ap()
    idf = nc.dram_tensor("idf", [P, P], F32, kind="ExternalInput").ap()
    out = nc.dram_tensor("out", [P, T], F32, kind="ExternalOutput").ap()

    chunks = [(c, c + 512) for c in range(0, RS, 512)]
    widths = []
    w = RS
    while w > E:
        widths.append(w // 2)
        w //= 2

    with tile.TileContext(nc) as tc, ExitStack() as ctx:
        constp = ctx.enter_context(tc.tile_pool(name="const", bufs=1))
        gatp = ctx.enter_context(tc.tile_pool(name="gat", bufs=GBUFS))
        idxp = ctx.enter_context(tc.tile_pool(name="idx", bufs=8))
        bigp = ctx.enter_context(tc.tile_pool(name="big", bufs=1))
        workp = ctx.enter_context(tc.tile_pool(name="work", bufs=2))
        psp = ctx.enter_context(tc.tile_pool(name="ps", bufs=1, space="PSUM"))
        psp2 = ctx.enter_context(tc.tile_pool(name="ps2", bufs=1, space="PSUM"))

        with tc.tile_critical():
            nc.gpsimd.load_library(library_config.mlp)

        ident = constp.tile([P, P], FP8)
        nc.sync.dma_start(out=ident[:], in_=id8[:])
        identf = constp.tile([P, P], F32, tag="identf")
        nc.sync.dma_start(out=identf[:], in_=idf[:])
        watt_sb = constp.tile([E, E], F32)
        nc.sync.dma_start(out=watt_sb[:], in_=watt[:])
        wv_sb = constp.tile([P, E], F32)
        nc.sync.dma_start(out=wv_sb[:], in_=wv[:])
        sc_sb = constp.tile([P, 2], F32)
        nc.sync.dma_start(out=sc_sb[:], in_=sc[:])
        res_sb = constp.tile([P, T], F32)

        for t in range(T):
            psum_S = psp.tile([P, RS], F32, tag="psum_S")
            d64 = bigp.tile([P, RS], F32, tag="d64")
            e1d = workp.tile([P, F], F32, tag="e1d")
            for h in range(HT):
                idx = idxp.tile([P, NI16], I16, tag="idx")
                slab = (t * HT + h) * P
                nc.sync.dma_start(out=idx[:], in_=xg[slab : slab + P, :])
                g = gatp.tile([P, FH * ROWB], FP8, tag="g")
                nc.gpsimd.dma_gather(
                    out_ap=g[:].rearrange("p (j r) -> p j r", r=ROWB),
                    in_ap=tfs[t // TPT][:],
                    idxs_ap=idx[:],
                    num_idxs=NI,
                    num_idxs_reg=NI,
                    elem_size=ROWB,
                    single_packet=False,
                )
                for jl in range(FH):
                    j = h * FH + jl
                    base = jl * ROWB
                    for c0, c1 in chunks:
                        nc.tensor.matmul(
                            out=psum_S[:, c0:c1],
                            lhsT=ident[:],
                            rhs=g[:, base + c0 : base + c1],
                            start=(j == 0),
                            stop=(j == F - 1),
                        )
                # own-field (diagonal) e2 strips: chunk jl holds field
                # j = h*FH + jl, strip at byte jl*ROWB + E*j
                gap = g[:]
                gd = bass.AP(
                    gap.tensor,
                    gap.offset + E * FH * h,
                    [[FH * ROWB, P], [ROWB + E, FH], [1, E]],
                )
                nc.scalar.copy(
                    out=d64[:, h * FH * E : (h + 1) * FH * E].rearrange(
                        "p (i v) -> p i v", v=E
                    ),
                    in_=gd,
                )
                # e1 diagonal (f32 tail of the row)
                gf = gap.bitcast(F32)
                ge1 = bass.AP(
                    gf.tensor,
                    gf.offset + E2B // 4 + FH * h,
                    [[FH * ROWB // 4, P], [ROWB // 4 + 1, FH]],
                )
                nc.scalar.copy(out=e1d[:, h * FH : (h + 1) * FH], in_=ge1)

            # ---- phase 2 ----
            prod = bigp.tile([P, RS], F32, tag="prod")
            nc.vector.scalar_tensor_tensor(
                out=prod[:], in0=psum_S[:], scalar=1.0 / (S_E2 * S_E2),
                in1=d64[:], op0=mybir.AluOpType.mult, op1=mybir.AluOpType.mult,
            )
            f = prod
            for w2 in widths:
                nf = bigp.tile([P, w2], F32, tag=f"fold{w2}")
                nc.vector.tensor_tensor(
                    out=nf[:], in0=f[:, :w2], in1=f[:, w2 : 2 * w2],
                    op=mybir.AluOpType.add,
                )
                f = nf
            second = f[:, 0:E]

            first = workp.tile([P, 1], F32, tag="first")
            nc.vector.tensor_reduce(
                out=first[:], in_=e1d[:], axis=mybir.AxisListType.X,
                op=mybir.AluOpType.add,
            )

            # ---- attention + output ----
            psum_T = psp2.tile([E, P], F32, tag="psum_T")
            nc.tensor.transpose(out=psum_T[:], in_=second, identity=identf[:])
            secT = workp.tile([E, P], F32, tag="secT")
            nc.vector.tensor_copy(out=secT[:], in_=psum_T[:])
            psum_L = psp2.tile([P, E], F32, tag="psum_L")
            nc.tensor.matmul(
                out=psum_L[:], lhsT=secT[:], rhs=watt_sb[:], start=True, stop=True
            )
            nmx = workp.tile([P, 1], F32, tag="nmx")
            nc.vector.tensor_reduce(
                out=nmx[:], in_=psum_L[:], axis=mybir.AxisListType.X,
                op=mybir.AluOpType.max, negate=True,
            )
            expv = workp.tile([P, E], F32, tag="expv")
            sume = workp.tile([P, 1], F32, tag="sume")
            nc.scalar.activation(
                out=expv[:], in_=psum_L[:],
                func=mybir.ActivationFunctionType.Exp,
                bias=nmx[:, 0:1], scale=1.0, accum_out=sume[:],
            )
            rin = workp.tile([P, 1], F32, tag="rin")
            nc.vector.reciprocal(out=rin[:], in_=sume[:])
            po = workp.tile([P, E], F32, tag="po")
            nc.vector.tensor_tensor(
                out=po[:], in0=expv[:], in1=second, op=mybir.AluOpType.mult
            )
            pw = workp.tile([P, E], F32, tag="pw")
            nc.vector.tensor_tensor(
                out=pw[:], in0=po[:], in1=wv_sb[:], op=mybir.AluOpType.mult
            )
            s2 = workp.tile([P, 1], F32, tag="s2")
            nc.vector.tensor_reduce(
                out=s2[:], in_=pw[:], axis=mybir.AxisListType.X,
                op=mybir.AluOpType.add,
            )
            fo = workp.tile([P, 1], F32, tag="fo")
            nc.vector.scalar_tensor_tensor(
                out=fo[:], in0=first[:], scalar=sc_sb[:, 0:1], in1=sc_sb[:, 1:2],
                op0=mybir.AluOpType.mult, op1=mybir.AluOpType.add,
            )
            nc.vector.scalar_tensor_tensor(
                out=res_sb[:, t : t + 1], in0=s2[:], scalar=rin[:, 0:1], in1=fo[:],
                op0=mybir.AluOpType.mult, op1=mybir.AluOpType.add,
            )

        nc.sync.dma_start(out=out[:], in_=res_sb[:])

    nc.compile()
    return nc


def _host_prep(x, e1, e2, W_att, W_out, b_out, n_tables):
    TPT = T // n_tables
    e2s = np.clip(e2.transpose(1, 0, 2).reshape(V, F * E) * S_E2, -448, 448)
    e2b = e2s.astype(NP_FP8).view(np.uint8)
    e1b = (
        np.ascontiguousarray(e1.T.astype(np.float32)).view(np.uint8).reshape(V, 4 * F)
    )
    tfull = np.zeros((V, ROWB), np.uint8)
    tfull[:, :E2B] = e2b
    tfull[:, E2B : E2B + 4 * F] = e1b

    xs = np.ascontiguousarray(x).astype(np.int64)
    watt = np.ascontiguousarray(W_att.astype(np.float32))
    wvec = np.broadcast_to(W_out[1:, 0].astype(np.float32)[None, :], (P, E)).copy()
    scv = np.broadcast_to(
        np.array([W_out[0, 0], b_out[0]], dtype=np.float32)[None, :], (P, 2)
    ).copy()
    id8 = np.eye(P, dtype=NP_FP8)
    idf = np.eye(P, dtype=np.float32)

    in_maps = []
    for c in range(N_CORES):
        xc = xs[c * BC : (c + 1) * BC]
        m = {"watt": watt, "wv": wvec, "sc": scv, "id8": id8, "idf": idf}
        xg = np.zeros((T * HT * P, NI16), np.int16)
        for tb in range(n_tables):
            xh = xc[tb * TPT * P : (tb + 1) * TPT * P]
            uniq, inv = np.unique(xh, return_inverse=True)
            if len(uniq) > min(NRT_PAD, 32767):
                return None  # caller retries with more tables
            tfh = np.zeros((NRT_PAD, ROWB), np.uint8)
            tfh[: len(uniq)] = tfull[uniq]
            m[f"tf{tb}"] = tfh.view(NP_FP8)
            xr = inv.reshape(TPT * P, F).astype(np.int16)
            for tl in range(TPT):
                t = tb * TPT + tl
                xt = xr[tl * P : (tl + 1) * P, :]
                for h in range(HT):
                    lst = xt[:, h * FH : (h + 1) * FH].T.ravel()
                    wrapped = lst.reshape(NI16, 16).T
                    slab = (t * HT + h) * P
                    for kk in range(8):
                        xg[slab + 16 * kk : slab + 16 * (kk + 1), :] = wrapped
        m["xg"] = xg
        in_maps.append(m)
    return in_maps


def kernel(x, e1, e2, W_att, W_out, b_out):
    global LAST_EXEC_TIME_NS
    from concourse.bass_utils import run_bass_kernel_spmd

    x = np.asarray(x)
    e1 = np.asarray(e1, dtype=np.float32)
    e2 = np.asarray(e2, dtype=np.float32)
    W_att = np.asarray(W_att, dtype=np.float32)
    W_out = np.asarray(W_out, dtype=np.float32)
    b_out = np.asarray(b_out, dtype=np.float32)

    n_tables = 2
    in_maps = _host_prep(x, e1, e2, W_att, W_out, b_out, n_tables)
    if in_maps is None:  # pathological id distribution; finer vocab split
        n_tables = 4
        in_maps = _host_prep(x, e1, e2, W_att, W_out, b_out, n_tables)
        assert in_maps is not None, "per-quarter unique ids exceed int16 range"

    nc = _build(n_tables)

    trace = bool(int(os.environ.get("AFM_TRACE", "0")))
    if not trace:
        # NTFF profiling needs the antenv.axon_hooks shim; without it the
        # trace path raises. Make plain runs immune to a stray BASS_TRACE.
        os.environ.setdefault("BASS_NEVER_TRACE", "1")
    res = run_bass_kernel_spmd(
        nc, in_maps, core_ids=list(range(N_CORES)), trace=trace
    )
    LAST_EXEC_TIME_NS = res.exec_time_ns

    outs = []
    for c in range(N_CORES):
        o = res.results[c]["out"]  # (P, T); col t = tile t, row p = sample
        outs.append(np.asarray(o).T.reshape(-1, 1))
    return np.concatenate(outs, axis=0).astype(np.float32)
